# revision 1
# baseline (speedup 1.0000x reference)
"""Multi-head attention (B=4, T=2048, D=1024, H=16) on 8 Trainium2 NeuronCores.

Sharding: core = (batch, head-group): b = core // 2, g = core % 2.
Each core computes heads [g*8, g*8+8) of batch b:
  - Q/K/V projections as fp8e4m3 DoubleRow residual matmuls: x and W ship as
    host-quantized (hi, lo) pairs; hi@hi runs over k-tile pairs and one
    cross-term instruction per k-tile adds w_hi@x_lo + w_lo@x_hi (only lo@lo
    is dropped), at 0.75x the bf16 PE cost and ~bf16 accuracy.  W is
    prescaled by 64 so its lo parts clear e4m3's subnormal floor; the scale
    is undone in the exp (q,k) and the V psum drain (v).
  - scores transposed: S.T tile = K_h @ Q_h.T (bf16, K=64) into alternating
    3-bank A/B PSUM slots; exp on ScalarE per [128, 3, 512] window
    (plus one 1-kt window), scale = 1/(sqrt(64)*64^2), no max subtraction
  - PV flipped: lhsT = P^T subtile (stationary), rhs = [V_h | 1] (moving,
    N=65) -> O natural [q, hd] per 128-query subtile with row-sums in col
    64; normalize on DVE (per-partition scalar), XBAR DMA transpose back to
    o_sb's [hd, q] layout (no PE broadcast/transpose rows)
  - partial output projection yT_g = Wo[:, g].T-contraction, bf16 out
Host: y[b] = (yT_part[2b] + yT_part[2b+1]).T + bo.

Scheduling: ScalarE's exp stream is the near-bottleneck, so score windows
are emitted at ACT's drain rate and every other PE chain (projections, PV,
output projection) is queued as a small piece popped between windows under
a credit budget.  Iterations walk (pair, chunk) anti-diagonals so the V
projection (early) and the per-chunk output projections (late) spread over
interleaved iterations; single-buffered PSUM slots alternate users so
chain drains overlap; the final output chunk parks its ki 0-2 partials in
the freed score slots while the last PV normalizes.

Self-contained: hardcodes all shapes; requires only concourse (bass) + numpy.
"""

import numpy as np

B, T, D = 4, 2048, 1024
H, HD = 16, 64
HG, DG = 8, 512          # heads / feature columns per core
NCORES = 8
P = 128
KD = D // P              # 8  k-tiles over model dim
MQ = DG // P             # 4  partition tiles of qT/kT/oT (one per head pair)
TK = T // P              # 16 key tiles
TQC = 512                # query-chunk (= one fp32 PSUM bank)
NC2 = T // TQC           # 4  query chunks
VW = HD + 1              # V columns per head incl. ones column
SCALE = 0.125            # 1/sqrt(HD)
WS = 64.0                # host-side weight prescale: keeps the fp8 residual
                         # (lo) parts of W ~ N(0, 1/1024) above e4m3's
                         # subnormal floor; undone via the exp scale (q,k)
                         # and the V psum->sbuf copy
SCALE_E = SCALE / (WS * WS)

_CACHE: dict = {}


def _emit(tc, aps, dbg=None, reps=1):
    import concourse.bass as bass  # noqa: F401
    from concourse import mybir

    nc = tc.nc
    dt = mybir.dt
    f32, bf16 = dt.float32, dt.bfloat16
    AF = mybir.ActivationFunctionType
    xT, wq, wk, wv, wo, bq, bk, bv, yT = (
        aps["xT"], aps["wq"], aps["wk"], aps["wv"], aps["wo"],
        aps["bq"], aps["bk"], aps["bv"], aps["yT"],
    )

    from collections import deque
    from contextlib import ExitStack

    with ExitStack() as ctx:
        const = ctx.enter_context(tc.tile_pool(name="const", bufs=1))
        persist = ctx.enter_context(tc.tile_pool(name="persist", bufs=1))
        xw = ctx.enter_context(tc.tile_pool(name="xw", bufs=1))
        ptp = ctx.enter_context(tc.tile_pool(name="ptp", bufs=4))
        opp = ctx.enter_context(tc.tile_pool(name="opp", bufs=4))
        yop = ctx.enter_context(tc.tile_pool(name="yop", bufs=6))
        nrm = ctx.enter_context(tc.tile_pool(name="nrm", bufs=6))
        # PSUM: 3(A) + 3(B) + 1(proj) + 1(pv) = 8 banks exactly
        scpsA = ctx.enter_context(tc.tile_pool(name="scpsA", bufs=1, space="PSUM"))
        scpsB = ctx.enter_context(tc.tile_pool(name="scpsB", bufs=1, space="PSUM"))
        qkvps = ctx.enter_context(tc.tile_pool(name="qkvps", bufs=1, space="PSUM"))
        pvps = ctx.enter_context(tc.tile_pool(name="pvps", bufs=1, space="PSUM"))

        # ---- persistent SBUF ----
        q_sb = persist.tile([P, MQ, T], bf16)
        k_sb = persist.tile([P, MQ, T], bf16)
        v_sb = persist.tile([P, TK, HG * VW], bf16)
        o_sb = persist.tile([P, MQ, T], bf16)
        v4d = v_sb.rearrange("p t (h c) -> p t h c", h=HG)
        nc.vector.memset(v4d[:, :, :, HD : HD + 1], 1.0)

        # ---- input DMAs ----------------------------------------------------
        # x and the q/k/v weights arrive as fp8 (hi, lo) residual pairs for
        # DoubleRow matmuls.  x dim2 order is (lo, hi); w dim2 is (hi, lo):
        # the cross-term DoubleRow instruction then contracts
        # w_hi.T@x_lo + w_lo.T@x_hi with natural slices.
        f8 = dt.float8e4
        x_sb = xw.tile([P, NC2, KD, 2, TQC], f8)
        wq_sb = xw.tile([P, MQ, KD, 2, P], f8)
        wk_sb = xw.tile([P, MQ, KD, 2, P], f8)
        wv_sb = xw.tile([P, KD, 2, DG], f8)
        bv_sb = xw.tile([1, DG], bf16)
        wo_sb = const.tile([P, MQ, D], bf16)
        bq_sb = const.tile([P, MQ], f32)
        bk_sb = const.tile([P, MQ], f32)
        # Single queue, critical-path order: wk then x chunk-slab 0 then the
        # m-tile-0 slice of wq unblock the first score windows ~8us in; the
        # remaining x slabs arrive one window group ahead of the k chains
        # that need them.  (A second queue would not help: HWDGE dispatch and
        # the DMA engines are shared resources and queue-local order is lost.)
        nc.sync.dma_start(out=wk_sb[:, 0], in_=wk[:, 0])
        nc.sync.dma_start(out=x_sb[:, 0], in_=xT[:, 0])
        nc.sync.dma_start(out=wq_sb[:, 0], in_=wq[:, 0])
        nc.sync.dma_start(out=bq_sb, in_=bq)
        nc.sync.dma_start(out=bk_sb, in_=bk)
        nc.sync.dma_start(out=bv_sb, in_=bv)
        nc.sync.dma_start(out=wk_sb[:, 1:], in_=wk[:, 1:])
        nc.sync.dma_start(out=x_sb[:, 1], in_=xT[:, 1])
        nc.sync.dma_start(out=wv_sb, in_=wv)
        nc.sync.dma_start(out=x_sb[:, 2], in_=xT[:, 2])
        nc.sync.dma_start(out=x_sb[:, 3], in_=xT[:, 3])
        nc.sync.dma_start(out=wq_sb[:, 1:], in_=wq[:, 1:])
        nc.sync.dma_start(out=wo_sb, in_=wo)
        ones_sb = xw.tile([1, P], bf16)
        nc.vector.memset(ones_sb, 1.0)
        DR = mybir.MatmulPerfMode.DoubleRow

        # p-state warmup: keep the PE continuously busy with junk matmuls
        # until the first projection chain's inputs land, so real work runs
        # at full clock instead of through the p-state ramp.
        warm_ps = qkvps.tile([P, HD], f32, tag="qkv", name="warm")
        for _ in range(100):
            nc.tensor.matmul(warm_ps, ones_sb, ones_sb[:, 0:HD], start=True, stop=True)

        # ---- filler FIFO + credit pump -------------------------------------
        # The exp() stream on ScalarE is the near-bottleneck; score windows
        # are emitted at ACT's drain rate and all other PE work (projection
        # chains, PV chains, output-projection chains) is queued as small
        # "filler" pieces popped between windows so the PE never waits on a
        # PSUM slot while ACT catches up.
        fifo = deque()  # (label, est_pe_ns, emit_fn)
        done = set()
        state = {"credit": 0.0}

        def piece(label, est, fn):
            fifo.append((label, est, fn))

        def pop_one():
            label, est, fn = fifo.popleft()
            fn()
            done.add(label)
            state["credit"] -= est

        def pump(add):
            state["credit"] = min(state["credit"] + add, 2600.0)
            while fifo and state["credit"] > 0:
                pop_one()

        def need(*labels):
            """Emit required pieces.  qk chains depend on nothing queued
            before them (DMA inputs, disjoint outputs), so they may jump the
            queue instead of dragging the whole FIFO prefix into a score
            window; anything else drains in order."""
            want = set(labels) - done
            for lbl in [w for w in want if w[0] == "qk"]:
                for idx, (l2, est, fn) in enumerate(fifo):
                    if l2 == lbl:
                        del fifo[idx]
                        fn()
                        done.add(lbl)
                        state["credit"] -= est
                        break
            want -= done
            while want:
                assert fifo, f"missing pieces: {want}"
                pop_one()
                want -= done

        # ---- work pieces ---------------------------------------------------
        def qk_chain(w_sb, b_col, dst, mt, n):
            """fp8 DoubleRow residual projection: hi@hi over k-tile pairs,
            then per-k-tile cross terms (w_hi@x_lo + w_lo@x_hi); only the
            lo@lo term is dropped (~1e-3 relative)."""
            ps = qkvps.tile([P, TQC], f32, tag="qkv", name="ps_qkv")
            for kp in range(KD // 2):
                nc.tensor.matmul(
                    ps,
                    w_sb[:, mt, 2 * kp : 2 * kp + 2, 0, :],
                    x_sb[:, n, 2 * kp : 2 * kp + 2, 1, :],
                    start=(kp == 0),
                    stop=False,
                    perf_mode=DR,
                )
            for ki in range(KD):
                nc.tensor.matmul(
                    ps,
                    w_sb[:, mt, ki, :, :],
                    x_sb[:, n, ki, :, :],
                    start=False,
                    stop=(ki == KD - 1),
                    perf_mode=DR,
                )
            nc.vector.tensor_scalar_add(
                dst[:, mt, n * TQC : (n + 1) * TQC], ps, b_col[:, mt : mt + 1]
            )

        def queue_qk(which, mt, n):
            w_sb, b_col, dst = {
                "k": (wk_sb, bk_sb, k_sb),
                "q": (wq_sb, bq_sb, q_sb),
            }[which]
            piece(
                ("qk", which, mt, n),
                1280,
                lambda w=w_sb, b=b_col, d=dst, mt=mt, n=n: qk_chain(w, b, d, mt, n),
            )

        def v_chain(t):
            """DoubleRow residual scheme with x stationary; the psum->sbuf
            copy divides out the host-side weight prescale and the bias rides
            a K=1 ones matmul.  Chains alternate between the two 1-bank psum
            pools so consecutive chains overlap each other's DVE drain."""
            pool, tg = ((qkvps, "qkv"), (pvps, "pv"))[t % 2]
            ps = pool.tile([P, DG], f32, tag=tg, name="ps_v")
            n, ts = t // (TQC // P), (t % (TQC // P)) * P
            for kp in range(KD // 2):
                nc.tensor.matmul(
                    ps,
                    x_sb[:, n, 2 * kp : 2 * kp + 2, 1, ts : ts + P],
                    wv_sb[:, 2 * kp : 2 * kp + 2, 0, :],
                    start=(kp == 0),
                    stop=False,
                    perf_mode=DR,
                )
            for ki in range(KD):
                nc.tensor.matmul(
                    ps,
                    x_sb[:, n, ki, :, ts : ts + P],
                    wv_sb[:, ki, :, :],
                    start=False,
                    stop=False,
                    perf_mode=DR,
                )
            nc.tensor.matmul(ps, ones_sb, bv_sb, start=False, stop=True)
            nc.vector.tensor_scalar_mul(
                v4d[:, t, :, 0:HD], ps.rearrange("p (h c) -> p h c", h=HG), 1.0 / WS
            )

        def queue_v():
            for t in range(TK):
                piece(("v", t), 1493, lambda t=t: v_chain(t))

        # Flipped PV for head h = 2p + i: lhsT = P^T subtile (stationary),
        # rhs = [V_h | 1] (moving, N = 65) so each 128-query subtile
        # accumulates O natural [q, hd] plus its softmax row-sums in column
        # 64.  Normalize on DVE (row-sums live on the free dim); after both
        # heads, XBAR DMA transposes put O back into o_sb's [hd, q] layout
        # without touching the PE.
        pvstate = {}

        def pv_chain(p, c, i, qs, pt):
            h = 2 * p + i
            if (i, qs) == (0, 0):
                pvstate["opr"] = opp.tile([P, MQ, 2, HD], bf16, name="opair")
            if qs == 0:
                pvstate["pv"] = pvps.tile([P, MQ, VW], f32, tag="pv", name="pv")
            pv = pvstate["pv"]
            for tk in range(TK):
                nc.tensor.matmul(
                    pv[:, qs, :],
                    pt[:, tk, qs * P : (qs + 1) * P],
                    v_sb[:, tk, h * VW : (h + 1) * VW],
                    start=(tk == 0),
                    stop=(tk == TK - 1),
                )
            opr = pvstate["opr"]
            rc = nrm.tile([P, 1], f32, name="rc")
            nc.vector.reciprocal(rc, pv[:, qs, HD : HD + 1])
            nc.vector.tensor_scalar_mul(opr[:, qs, i, :], pv[:, qs, 0:HD], rc)
            if i == 1:
                # this query-subtile now has both heads normalized: transpose
                # it back immediately rather than after the whole head drains
                tq0 = c * TQC
                nc.sync.dma_start_transpose(
                    out=o_sb[:, p, tq0 + qs * P : tq0 + (qs + 1) * P],
                    in_=opr[:, qs, :, :],
                )

        def pv_labels(p, c, i):
            return [("pv", p, c, i, qs) for qs in range(MQ)]

        def queue_pv_head(p, c, i, pts):
            for qs in range(MQ):
                piece(
                    ("pv", p, c, i, qs),
                    433,
                    lambda p=p, c=c, i=i, qs=qs, pt=pts[i]: pv_chain(p, c, i, qs, pt),
                )

        def o_chain(c, j, pool=None, tag=None, drain=None):
            tq0 = c * TQC
            if pool is None:
                pool, tag = ((qkvps, "qkv"), (pvps, "pv"))[j % 2]
            ys = pool.tile([P, TQC], f32, tag=tag, name="ys")
            for ki in range(MQ):
                nc.tensor.matmul(
                    ys,
                    wo_sb[:, ki, j * P : (j + 1) * P],
                    o_sb[:, ki, tq0 : tq0 + TQC],
                    start=(ki == 0),
                    stop=(ki == MQ - 1),
                )
            yo = yop.tile([P, TQC], bf16, name="yo")
            if drain == "act":
                nc.scalar.copy(yo, ys)
            else:
                nc.vector.tensor_copy(yo, ys)
            nc.sync.dma_start(out=yT[:, j, tq0 : tq0 + TQC], in_=yo)

        def queue_oproj(c):
            for j in range(D // P):
                piece(("oproj", c, j), 853, lambda c=c, j=j: o_chain(c, j))

        # ---- score windows -------------------------------------------------
        # Two 3-bank PSUM slots, windows [3,3,3,3,3,1] strictly alternating
        # A/B: larger exp() instructions amortize ScalarE's fixed
        # per-instruction cost and no window ever reuses the slot of the
        # immediately preceding one, so ACT never waits a matmul+2-sem
        # round-trip at a window boundary.
        WINDOWS = (
            (scpsA, "scA", 3, 825.0),
            (scpsB, "scB", 3, 825.0),
            (scpsA, "scA", 3, 825.0),
            (scpsB, "scB", 3, 825.0),
            (scpsA, "scA", 3, 825.0),
            (scpsB, "scB", 1, 399.0),
        )

        def scores_exp_head(p, c, i, pt, pt_guard=(), boost=1.0):
            hb = i * HD
            tq0 = c * TQC
            kt0 = 0
            for pool, tg, wn, credit in WINDOWS:
                n_lo = (kt0 * P) // TQC
                n_hi = ((kt0 + wn) * P - 1) // TQC
                need(
                    *[("qk", "k", p, n2) for n2 in range(n_lo, n_hi + 1)],
                    ("qk", "q", p, c),
                )
                scs = pool.tile([P, wn, TQC], f32, tag=tg, name=tg)
                for u in range(wn):
                    tk = kt0 + u
                    nc.tensor.matmul(
                        scs[:, u, :],
                        k_sb[hb : hb + HD, p, tk * P : (tk + 1) * P],
                        q_sb[hb : hb + HD, p, tq0 : tq0 + TQC],
                        start=True,
                        stop=True,
                    )
                if kt0 == 0 and pt_guard:
                    # the exp below reuses the pt slot read by a PV two
                    # iterations back: force that PV out after this window's
                    # matmuls (which don't touch pt) rather than before
                    need(*pt_guard)
                nc.scalar.activation(pt[:, kt0 : kt0 + wn, :], scs, AF.Exp, scale=SCALE_E)
                kt0 += wn
                pump(credit * boost)

        # ---- schedule: pair-outer, chunk-inner -----------------------------
        if reps > 1:
            loop_cm = tc.For_i(0, reps, 1)
            loop_cm.__enter__()

        fifo.clear()
        done.clear()
        state["credit"] = 0.0

        # prelude: first score window needs k m-tile 0 (chunk 0) + q chunk 0
        qk_chain(wk_sb, bk_sb, k_sb, 0, 0)
        done.add(("qk", "k", 0, 0))
        qk_chain(wq_sb, bq_sb, q_sb, 0, 0)
        done.add(("qk", "q", 0, 0))
        for n in range(1, NC2):
            queue_qk("k", 0, n)

        # Anti-diagonal iteration order: row 0's V-projection overload and
        # row 3's output-projection load spread over interleaved iterations
        # of the other rows instead of saturating one row while ScalarE
        # starves.
        ORDER = [
            (p, s - p)
            for s in range(MQ + NC2 - 1)
            for p in range(max(0, s - NC2 + 1), min(MQ - 1, s) + 1)
        ]
        hist = []
        oproj_queued = set()
        for p, c in ORDER:
            # Queue this iteration's filler.  PV chains for the previous
            # pair are sandwiched between qkv-bank chains so the
            # single-buffered PSUM slots always have covering PE work
            # between their DVE drain and reuse.
            qkv_new = []
            if p == 0 and c < NC2 - 1:
                qkv_new.append(("q", 0, c + 1))
            if p < MQ - 1:
                qkv_new.append(("q", p + 1, c))
            # k for row s+1 (first used on the next diagonal) spread across
            # this diagonal's iterations; scores need a pair's k over ALL
            # key chunks at its first iteration
            s = p + c
            if s + 1 < MQ:
                d_lo = max(0, s - NC2 + 1)
                n_iters = min(MQ - 1, s) - d_lo + 1
                for n in range(NC2):
                    if n % n_iters == p - d_lo:
                        qkv_new.append(("k", s + 1, n))
            if hist:
                queue_pv_head(*hist[-1][:2], 0, hist[-1][2])
            if qkv_new:
                queue_qk(*qkv_new.pop(0))
            if hist:
                queue_pv_head(*hist[-1][:2], 1, hist[-1][2])
            for spec in qkv_new:
                queue_qk(*spec)
            if p == 0 and c == 0:
                queue_v()
            # output projection for any chunk whose last pair's PV (and thus
            # its o_sb transposes) is queued above or in an earlier iteration
            for cc in range(NC2):
                if cc not in oproj_queued and (MQ - 1, cc) in [h[:2] for h in hist]:
                    oproj_queued.add(cc)
                    queue_oproj(cc)
            pts = [
                ptp.tile([P, TK, TQC], bf16, tag="pt", name="pt0"),
                ptp.tile([P, TK, TQC], bf16, tag="pt", name="pt1"),
            ]
            g0 = pv_labels(*hist[-2][:2], 0) if len(hist) >= 2 else ()
            g1 = pv_labels(*hist[-2][:2], 1) if len(hist) >= 2 else ()
            # early iterations run ACT behind the input DMAs anyway, so the
            # windows can afford extra filler to pre-drain the V backlog
            boost = 1.4 if len(hist) < 6 else 1.0
            scores_exp_head(p, c, 0, pts[0], pt_guard=g0, boost=boost)
            if (p, c) == ORDER[-1]:
                # last iteration: its own head-0 PV can pump under the
                # head-1 score windows
                queue_pv_head(p, c, 0, pts)
            scores_exp_head(p, c, 1, pts[1], pt_guard=g1, boost=boost)
            hist.append((p, c, pts))

        # tail: last pair's PV, then the final output-projection chunk.
        # The first four chains park their ki 0-2 partial sums in the four
        # psum pools (score slots are free after the last exp) so they
        # execute under the final PV's normalize/transpose flight; only the
        # ki=3 matmuls wait for the last pair's o_sb transposes.
        p, c, pts = hist[-1]
        while fifo:
            pop_one()
        TAILP = ((qkvps, "qkv"), (scpsA, "scA"), (pvps, "pv"), (scpsB, "scB"))
        tq0 = (NC2 - 1) * TQC
        parked = [None] * 4

        def park(j):
            pool, tag = TAILP[j]
            ys = pool.tile([P, TQC], f32, tag=tag, name="ys")
            for ki in range(MQ - 1):
                nc.tensor.matmul(
                    ys,
                    wo_sb[:, ki, j * P : (j + 1) * P],
                    o_sb[:, ki, tq0 : tq0 + TQC],
                    start=(ki == 0),
                    stop=False,
                )
            parked[j] = ys

        # the qkvps-slot partial depends only on long-finished state: run it
        # under the last head's exp drain before the final PV chains
        park(0)
        for qs in range(MQ):
            pv_chain(p, c, 1, qs, pts[1])
        for j in (1, 2, 3):
            park(j)
        for j in range(4):
            ys = parked[j]
            nc.tensor.matmul(
                ys,
                wo_sb[:, MQ - 1, j * P : (j + 1) * P],
                o_sb[:, MQ - 1, tq0 : tq0 + TQC],
                start=False,
                stop=True,
            )
            yo = yop.tile([P, TQC], bf16, name="yo")
            if j % 2:
                nc.scalar.copy(yo, ys)
            else:
                nc.vector.tensor_copy(yo, ys)
            nc.sync.dma_start(out=yT[:, j, tq0 : tq0 + TQC], in_=yo)
        for j in range(4, D // P):
            pool, tag = TAILP[j % 4]
            o_chain(NC2 - 1, j, pool=pool, tag=tag, drain=("act" if j % 2 else None))

        if reps > 1:
            loop_cm.__exit__(None, None, None)

        if dbg is not None:
            nc.sync.dma_start(out=dbg["q"], in_=q_sb)
            nc.sync.dma_start(out=dbg["k"], in_=k_sb)
            nc.sync.dma_start(out=dbg["v"], in_=v_sb)
            nc.sync.dma_start(out=dbg["o"], in_=o_sb)


def _build(debug=False, reps=1):
    import concourse.tile as tile
    from concourse import bacc, mybir

    dt = mybir.dt
    f32, bf16 = dt.float32, dt.bfloat16

    f8 = dt.float8e4
    nc = bacc.Bacc("TRN2", target_bir_lowering=False, debug=False)
    # inputs are host-preswizzled into partition-major layouts so every DMA
    # descriptor is a fat contiguous run; x/wq/wk/wv ship as fp8 (hi, lo)
    # residual pairs (same byte volume as bf16)
    aps = {
        "xT": nc.dram_tensor("xT", [P, NC2, KD, 2, TQC], f8, kind="ExternalInput").ap(),
        "wq": nc.dram_tensor("wq", [P, MQ, KD, 2, P], f8, kind="ExternalInput").ap(),
        "wk": nc.dram_tensor("wk", [P, MQ, KD, 2, P], f8, kind="ExternalInput").ap(),
        "wv": nc.dram_tensor("wv", [P, KD, 2, DG], f8, kind="ExternalInput").ap(),
        "wo": nc.dram_tensor("wo", [P, MQ, D], bf16, kind="ExternalInput").ap(),
        "bq": nc.dram_tensor("bq", [P, MQ], f32, kind="ExternalInput").ap(),
        "bk": nc.dram_tensor("bk", [P, MQ], f32, kind="ExternalInput").ap(),
        "bv": nc.dram_tensor("bv", [1, DG], bf16, kind="ExternalInput").ap(),
        "yT": nc.dram_tensor("yT", [P, D // P, T], bf16, kind="ExternalOutput").ap(),
    }

    dbg = None
    if debug:
        dbg = {
            "q": nc.dram_tensor("dbg_q", [P, MQ, T], bf16, kind="ExternalOutput").ap(),
            "k": nc.dram_tensor("dbg_k", [P, MQ, T], bf16, kind="ExternalOutput").ap(),
            "v": nc.dram_tensor(
                "dbg_v", [P, TK, HG * VW], bf16, kind="ExternalOutput"
            ).ap(),
            "o": nc.dram_tensor("dbg_o", [P, MQ, T], bf16, kind="ExternalOutput").ap(),
            "pt": nc.dram_tensor(
                "dbg_pt", [P, TK, TQC], bf16, kind="ExternalOutput"
            ).ap(),
        }

    with tile.TileContext(nc) as tc:
        _emit(tc, aps, dbg, reps=reps)
    nc.compile()
    return nc


def _get_nc():
    if "nc" not in _CACHE:
        _CACHE["nc"] = _build()
    return _CACHE["nc"]


def _shard_inputs(x, Wq, bq, Wk, bk, Wv, bv, Wo, bo):
    import ml_dtypes

    bf16 = ml_dtypes.bfloat16
    f8 = ml_dtypes.float8_e4m3
    f32 = np.float32

    def c(a, dtype):
        return np.ascontiguousarray(a).astype(dtype)

    def kp(a, kt):  # [kt*P, F] -> [P, kt, F] partition-major swizzle
        return a.reshape(kt, P, a.shape[-1]).transpose(1, 0, 2)

    def hilo(a, order):  # [P, kt, F] f32 -> [P, kt, 2, F] fp8 residual pair
        hi = a.astype(f8)
        lo = (a - hi.astype(f32)).astype(f8)
        pair = {"hilo": (hi, lo), "lohi": (lo, hi)}[order]
        return np.ascontiguousarray(np.stack(pair, axis=2))

    def chunk_major(a8):  # [P, KD, 2, T] -> [P, NC2, KD, 2, TQC]
        return np.ascontiguousarray(
            a8.reshape(P, KD, 2, NC2, TQC).transpose(0, 3, 1, 2, 4)
        )

    def mtile_major(a8):  # [P, KD, 2, DG] -> [P, MQ, KD, 2, P]
        return np.ascontiguousarray(
            a8.reshape(P, KD, 2, MQ, P).transpose(0, 3, 1, 2, 4)
        )

    x8 = {}  # per-batch, shared by the two head-group cores
    in_maps = []
    for core in range(NCORES):
        b, g = core // 2, core % 2
        hs = g * DG
        if b not in x8:
            x8[b] = chunk_major(hilo(kp(np.asarray(x[b], dtype=f32).T, KD), "lohi"))
        in_maps.append(
            {
                "xT": x8[b],
                "wq": mtile_major(hilo(kp(Wq[hs : hs + DG, :].T * WS, KD), "hilo")),
                "wk": mtile_major(hilo(kp(Wk[hs : hs + DG, :].T * WS, KD), "hilo")),
                "wv": hilo(kp(Wv[hs : hs + DG, :].T * WS, KD), "hilo"),
                "wo": c(kp(Wo[:, hs : hs + DG].T, MQ), bf16),
                "bq": c(bq[hs : hs + DG].reshape(MQ, P).T * WS, f32),
                "bk": c(bk[hs : hs + DG].reshape(MQ, P).T * WS, f32),
                "bv": c(bv[hs : hs + DG].reshape(1, DG) * WS, bf16),
            }
        )
    return in_maps


def _run(inputs, trace=False):
    from concourse import bass_utils

    nc = _get_nc()
    np_in = {k: np.asarray(v) for k, v in inputs.items()}
    in_maps = _shard_inputs(**np_in)
    res = bass_utils.run_bass_kernel_spmd(
        nc, in_maps, core_ids=list(range(NCORES)), trace=trace
    )
    bo = np_in["bo"].astype(np.float32)
    y = np.empty((B, T, D), dtype=np.float32)
    for b in range(B):
        acc = res.results[2 * b]["yT"].astype(np.float32) + res.results[
            2 * b + 1
        ]["yT"].astype(np.float32)  # [P, D/P, T]
        y[b] = acc.transpose(1, 0, 2).reshape(D, T).T + bo
    return y, res


def kernel(**inputs):
    y, _ = _run(inputs)
    return y



# revision 3
# speedup vs baseline: 1.0176x; 1.0176x over previous
"""Multi-head attention (B=4, T=2048, D=1024, H=16) on 8 Trainium2 NeuronCores.

Sharding: core = (batch, head-group): b = core // 2, g = core % 2.
Each core computes heads [g*8, g*8+8) of batch b:
  - Q/K/V projections as fp8e4m3 DoubleRow residual matmuls: x and W ship as
    host-quantized (hi, lo) pairs; hi@hi runs over k-tile pairs and one
    cross-term instruction per k-tile adds w_hi@x_lo + w_lo@x_hi (only lo@lo
    is dropped), at 0.75x the bf16 PE cost and ~bf16 accuracy.  W is
    prescaled by 64 so its lo parts clear e4m3's subnormal floor; the scale
    is undone in the exp (q,k) and the V psum drain (v).
  - scores transposed: S.T tile = K_h @ Q_h.T (bf16, K=64) into alternating
    3-bank A/B PSUM slots; exp on ScalarE per [128, 3, 512] window
    (plus one 1-kt window), scale = 1/(sqrt(64)*64^2), no max subtraction
  - PV flipped: lhsT = P^T subtile (stationary), rhs = [V_h | 1] (moving,
    N=65) -> O natural [q, hd] per 128-query subtile with row-sums in col
    64; normalize on DVE (per-partition scalar), XBAR DMA transpose back to
    o_sb's [hd, q] layout (no PE broadcast/transpose rows)
  - partial output projection yT_g = Wo[:, g].T-contraction, bf16 out
Host: y[b] = (yT_part[2b] + yT_part[2b+1]).T + bo.

Scheduling: ScalarE's exp stream is the near-bottleneck, so score windows
are emitted at ACT's drain rate and every other PE chain (projections, PV,
output projection) is queued as a small piece popped between windows under
a credit budget.  Iterations walk (pair, chunk) anti-diagonals so the V
projection (early) and the per-chunk output projections (late) spread over
interleaved iterations; single-buffered PSUM slots alternate users so
chain drains overlap; the final output chunk parks its ki 0-2 partials in
the freed score slots while the last PV normalizes.

Self-contained: hardcodes all shapes; requires only concourse (bass) + numpy.
"""

import numpy as np

B, T, D = 4, 2048, 1024
H, HD = 16, 64
HG, DG = 8, 512          # heads / feature columns per core
NCORES = 8
P = 128
KD = D // P              # 8  k-tiles over model dim
MQ = DG // P             # 4  partition tiles of qT/kT/oT (one per head pair)
TK = T // P              # 16 key tiles
TQC = 512                # query-chunk (= one fp32 PSUM bank)
NC2 = T // TQC           # 4  query chunks
VW = HD + 1              # V columns per head incl. ones column
SCALE = 0.125            # 1/sqrt(HD)
WS = 64.0                # host-side weight prescale: keeps the fp8 residual
                         # (lo) parts of W ~ N(0, 1/1024) above e4m3's
                         # subnormal floor; undone via the exp scale (q,k)
                         # and the V psum->sbuf copy
SCALE_E = SCALE / (WS * WS)

_CACHE: dict = {}


def _emit(tc, aps, dbg=None, reps=1):
    import concourse.bass as bass  # noqa: F401
    from concourse import mybir

    nc = tc.nc
    dt = mybir.dt
    f32, bf16 = dt.float32, dt.bfloat16
    AF = mybir.ActivationFunctionType
    xT, wq, wk, wv, wo, bq, bk, bv, yT = (
        aps["xT"], aps["wq"], aps["wk"], aps["wv"], aps["wo"],
        aps["bq"], aps["bk"], aps["bv"], aps["yT"],
    )

    from collections import deque
    from contextlib import ExitStack

    with ExitStack() as ctx:
        const = ctx.enter_context(tc.tile_pool(name="const", bufs=1))
        persist = ctx.enter_context(tc.tile_pool(name="persist", bufs=1))
        xw = ctx.enter_context(tc.tile_pool(name="xw", bufs=1))
        ptp = ctx.enter_context(tc.tile_pool(name="ptp", bufs=4))
        opp = ctx.enter_context(tc.tile_pool(name="opp", bufs=4))
        yop = ctx.enter_context(tc.tile_pool(name="yop", bufs=6))
        nrm = ctx.enter_context(tc.tile_pool(name="nrm", bufs=6))
        # PSUM: 3(A) + 3(B) + 1(proj) + 1(pv) = 8 banks exactly
        scpsA = ctx.enter_context(tc.tile_pool(name="scpsA", bufs=1, space="PSUM"))
        scpsB = ctx.enter_context(tc.tile_pool(name="scpsB", bufs=1, space="PSUM"))
        qkvps = ctx.enter_context(tc.tile_pool(name="qkvps", bufs=1, space="PSUM"))
        pvps = ctx.enter_context(tc.tile_pool(name="pvps", bufs=1, space="PSUM"))

        # ---- persistent SBUF ----
        q_sb = persist.tile([P, MQ, T], bf16)
        k_sb = persist.tile([P, MQ, T], bf16)
        v_sb = persist.tile([P, TK, HG * VW], bf16)
        o_sb = persist.tile([P, MQ, T], bf16)
        v4d = v_sb.rearrange("p t (h c) -> p t h c", h=HG)
        nc.vector.memset(v4d[:, :, :, HD : HD + 1], 1.0)

        # ---- input DMAs ----------------------------------------------------
        # x and the q/k/v weights arrive as fp8 (hi, lo) residual pairs for
        # DoubleRow matmuls.  x dim2 order is (lo, hi); w dim2 is (hi, lo):
        # the cross-term DoubleRow instruction then contracts
        # w_hi.T@x_lo + w_lo.T@x_hi with natural slices.
        f8 = dt.float8e4
        x_sb = xw.tile([P, NC2, KD, 2, TQC], f8)
        wq_sb = xw.tile([P, MQ, KD, 2, P], f8)
        wk_sb = xw.tile([P, MQ, KD, 2, P], f8)
        wv_sb = xw.tile([P, KD, 2, DG], f8)
        bv_sb = xw.tile([1, DG], bf16)
        wo_sb = const.tile([P, MQ, D], bf16)
        bq_sb = const.tile([P, MQ], f32)
        bk_sb = const.tile([P, MQ], f32)
        # Single queue, critical-path order: wk then x chunk-slab 0 then the
        # m-tile-0 slice of wq unblock the first score windows ~8us in; the
        # remaining x slabs arrive one window group ahead of the k chains
        # that need them.  (A second queue would not help: HWDGE dispatch and
        # the DMA engines are shared resources and queue-local order is lost.)
        nc.sync.dma_start(out=wk_sb[:, 0], in_=wk[:, 0])
        nc.sync.dma_start(out=x_sb[:, 0], in_=xT[:, 0])
        nc.sync.dma_start(out=wq_sb[:, 0], in_=wq[:, 0])
        nc.sync.dma_start(out=bq_sb, in_=bq)
        nc.sync.dma_start(out=bk_sb, in_=bk)
        nc.sync.dma_start(out=bv_sb, in_=bv)
        nc.sync.dma_start(out=wk_sb[:, 1:], in_=wk[:, 1:])
        nc.sync.dma_start(out=x_sb[:, 1], in_=xT[:, 1])
        nc.sync.dma_start(out=wv_sb, in_=wv)
        nc.sync.dma_start(out=x_sb[:, 2], in_=xT[:, 2])
        nc.sync.dma_start(out=x_sb[:, 3], in_=xT[:, 3])
        nc.sync.dma_start(out=wq_sb[:, 1:], in_=wq[:, 1:])
        nc.sync.dma_start(out=wo_sb, in_=wo)
        ones_sb = xw.tile([1, P], bf16)
        nc.vector.memset(ones_sb, 1.0)
        DR = mybir.MatmulPerfMode.DoubleRow

        # p-state warmup: keep the PE continuously busy with junk matmuls
        # until the first projection chain's inputs land, so real work runs
        # at full clock instead of through the p-state ramp.
        warm_ps = qkvps.tile([P, HD], f32, tag="qkv", name="warm")
        for _ in range(60):
            nc.tensor.matmul(warm_ps, ones_sb, ones_sb[:, 0:HD], start=True, stop=True)

        # ---- filler FIFO + credit pump -------------------------------------
        # The exp() stream on ScalarE is the near-bottleneck; score windows
        # are emitted at ACT's drain rate and all other PE work (projection
        # chains, PV chains, output-projection chains) is queued as small
        # "filler" pieces popped between windows so the PE never waits on a
        # PSUM slot while ACT catches up.
        fifo = deque()  # (label, est_pe_ns, emit_fn)
        done = set()
        state = {"credit": 0.0}

        def piece(label, est, fn):
            fifo.append((label, est, fn))

        def pop_one():
            label, est, fn = fifo.popleft()
            fn()
            done.add(label)
            state["credit"] -= est

        def pump(add):
            state["credit"] = min(state["credit"] + add, 2600.0)
            while fifo and state["credit"] > 0:
                pop_one()

        def need(*labels):
            """Emit required pieces.  qk chains depend on nothing queued
            before them (DMA inputs, disjoint outputs), so they may jump the
            queue instead of dragging the whole FIFO prefix into a score
            window; anything else drains in order."""
            want = set(labels) - done
            for lbl in [w for w in want if w[0] == "qk"]:
                for idx, (l2, est, fn) in enumerate(fifo):
                    if l2 == lbl:
                        del fifo[idx]
                        fn()
                        done.add(lbl)
                        state["credit"] -= est
                        break
            want -= done
            while want:
                assert fifo, f"missing pieces: {want}"
                pop_one()
                want -= done

        # ---- work pieces ---------------------------------------------------
        def qk_chain(w_sb, b_col, dst, mt, n):
            """fp8 DoubleRow residual projection: hi@hi over k-tile pairs,
            then per-k-tile cross terms (w_hi@x_lo + w_lo@x_hi); only the
            lo@lo term is dropped (~1e-3 relative)."""
            ps = qkvps.tile([P, TQC], f32, tag="qkv", name="ps_qkv")
            for kp in range(KD // 2):
                nc.tensor.matmul(
                    ps,
                    w_sb[:, mt, 2 * kp : 2 * kp + 2, 0, :],
                    x_sb[:, n, 2 * kp : 2 * kp + 2, 1, :],
                    start=(kp == 0),
                    stop=False,
                    perf_mode=DR,
                )
            for ki in range(KD):
                nc.tensor.matmul(
                    ps,
                    w_sb[:, mt, ki, :, :],
                    x_sb[:, n, ki, :, :],
                    start=False,
                    stop=(ki == KD - 1),
                    perf_mode=DR,
                )
            nc.vector.tensor_scalar_add(
                dst[:, mt, n * TQC : (n + 1) * TQC], ps, b_col[:, mt : mt + 1]
            )

        def queue_qk(which, mt, n):
            w_sb, b_col, dst = {
                "k": (wk_sb, bk_sb, k_sb),
                "q": (wq_sb, bq_sb, q_sb),
            }[which]
            piece(
                ("qk", which, mt, n),
                1280,
                lambda w=w_sb, b=b_col, d=dst, mt=mt, n=n: qk_chain(w, b, d, mt, n),
            )

        def v_chain(t):
            """DoubleRow residual scheme with x stationary; the psum->sbuf
            copy divides out the host-side weight prescale and the bias rides
            a K=1 ones matmul.  Chains alternate between the two 1-bank psum
            pools so consecutive chains overlap each other's DVE drain."""
            pool, tg = ((qkvps, "qkv"), (pvps, "pv"))[t % 2]
            ps = pool.tile([P, DG], f32, tag=tg, name="ps_v")
            n, ts = t // (TQC // P), (t % (TQC // P)) * P
            for kp in range(KD // 2):
                nc.tensor.matmul(
                    ps,
                    x_sb[:, n, 2 * kp : 2 * kp + 2, 1, ts : ts + P],
                    wv_sb[:, 2 * kp : 2 * kp + 2, 0, :],
                    start=(kp == 0),
                    stop=False,
                    perf_mode=DR,
                )
            for ki in range(KD):
                nc.tensor.matmul(
                    ps,
                    x_sb[:, n, ki, :, ts : ts + P],
                    wv_sb[:, ki, :, :],
                    start=False,
                    stop=False,
                    perf_mode=DR,
                )
            nc.tensor.matmul(ps, ones_sb, bv_sb, start=False, stop=True)
            nc.vector.tensor_scalar_mul(
                v4d[:, t, :, 0:HD], ps.rearrange("p (h c) -> p h c", h=HG), 1.0 / WS
            )

        def queue_v():
            for t in range(TK):
                piece(("v", t), 1493, lambda t=t: v_chain(t))

        # Flipped PV for head h = 2p + i: lhsT = P^T subtile (stationary),
        # rhs = [V_h | 1] (moving, N = 65) so each 128-query subtile
        # accumulates O natural [q, hd] plus its softmax row-sums in column
        # 64.  Normalize on DVE (row-sums live on the free dim); after both
        # heads, XBAR DMA transposes put O back into o_sb's [hd, q] layout
        # without touching the PE.
        pvstate = {}

        def pv_chain(p, c, i, qs, pt):
            h = 2 * p + i
            if (i, qs) == (0, 0):
                pvstate["opr"] = opp.tile([P, MQ, 2, HD], bf16, name="opair")
            if qs == 0:
                pvstate["pv"] = pvps.tile([P, MQ, VW], f32, tag="pv", name="pv")
            pv = pvstate["pv"]
            for tk in range(TK):
                nc.tensor.matmul(
                    pv[:, qs, :],
                    pt[:, tk, qs * P : (qs + 1) * P],
                    v_sb[:, tk, h * VW : (h + 1) * VW],
                    start=(tk == 0),
                    stop=(tk == TK - 1),
                )
            opr = pvstate["opr"]
            rc = nrm.tile([P, 1], f32, name="rc")
            nc.vector.reciprocal(rc, pv[:, qs, HD : HD + 1])
            nc.vector.tensor_scalar_mul(opr[:, qs, i, :], pv[:, qs, 0:HD], rc)
            if i == 1:
                # this query-subtile now has both heads normalized: transpose
                # it back immediately rather than after the whole head drains
                tq0 = c * TQC
                nc.sync.dma_start_transpose(
                    out=o_sb[:, p, tq0 + qs * P : tq0 + (qs + 1) * P],
                    in_=opr[:, qs, :, :],
                )

        def pv_labels(p, c, i):
            return [("pv", p, c, i, qs) for qs in range(MQ)]

        def queue_pv_head(p, c, i, pts):
            for qs in range(MQ):
                piece(
                    ("pv", p, c, i, qs),
                    433,
                    lambda p=p, c=c, i=i, qs=qs, pt=pts[i]: pv_chain(p, c, i, qs, pt),
                )

        def o_chain(c, j, pool=None, tag=None, drain=None):
            tq0 = c * TQC
            if pool is None:
                pool, tag = ((qkvps, "qkv"), (pvps, "pv"))[j % 2]
            ys = pool.tile([P, TQC], f32, tag=tag, name="ys")
            for ki in range(MQ):
                nc.tensor.matmul(
                    ys,
                    wo_sb[:, ki, j * P : (j + 1) * P],
                    o_sb[:, ki, tq0 : tq0 + TQC],
                    start=(ki == 0),
                    stop=(ki == MQ - 1),
                )
            yo = yop.tile([P, TQC], bf16, name="yo")
            if drain == "act":
                nc.scalar.copy(yo, ys)
            else:
                nc.vector.tensor_copy(yo, ys)
            nc.sync.dma_start(out=yT[:, j, tq0 : tq0 + TQC], in_=yo)

        def queue_oproj(c):
            for j in range(D // P):
                piece(("oproj", c, j), 853, lambda c=c, j=j: o_chain(c, j))

        # ---- score windows -------------------------------------------------
        # Two 3-bank PSUM slots, windows [3,3,3,3,2,2] strictly alternating
        # A/B: larger exp() instructions amortize ScalarE's fixed
        # per-instruction cost and no window ever reuses the slot of the
        # immediately preceding one.  The two trailing 2-tile windows keep
        # every window's ACT cover >= ~1040ns: a pool slot's next refill
        # needs sem + fill + sem (~890ns) after its previous exp ends, so a
        # trailing 1-tile window (612ns cover) would stall ACT ~380ns at
        # every head boundary.
        WINDOWS = (
            (scpsA, "scA", 3, 825.0),
            (scpsB, "scB", 3, 825.0),
            (scpsA, "scA", 3, 825.0),
            (scpsB, "scB", 3, 825.0),
            (scpsA, "scA", 2, 612.0),
            (scpsB, "scB", 2, 612.0),
        )

        def scores_exp_head(p, c, i, pt, pt_guard=(), boost=1.0):
            hb = i * HD
            tq0 = c * TQC
            kt0 = 0
            for pool, tg, wn, credit in WINDOWS:
                n_lo = (kt0 * P) // TQC
                n_hi = ((kt0 + wn) * P - 1) // TQC
                need(
                    *[("qk", "k", p, n2) for n2 in range(n_lo, n_hi + 1)],
                    ("qk", "q", p, c),
                )
                scs = pool.tile([P, wn, TQC], f32, tag=tg, name=tg)
                for u in range(wn):
                    tk = kt0 + u
                    nc.tensor.matmul(
                        scs[:, u, :],
                        k_sb[hb : hb + HD, p, tk * P : (tk + 1) * P],
                        q_sb[hb : hb + HD, p, tq0 : tq0 + TQC],
                        start=True,
                        stop=True,
                    )
                if kt0 == 0 and pt_guard:
                    # the exp below reuses the pt slot read by a PV two
                    # iterations back: force that PV out after this window's
                    # matmuls (which don't touch pt) rather than before
                    need(*pt_guard)
                nc.scalar.activation(pt[:, kt0 : kt0 + wn, :], scs, AF.Exp, scale=SCALE_E)
                kt0 += wn
                pump(credit * boost)

        # ---- schedule: pair-outer, chunk-inner -----------------------------
        if reps > 1:
            loop_cm = tc.For_i(0, reps, 1)
            loop_cm.__enter__()

        fifo.clear()
        done.clear()
        state["credit"] = 0.0

        # prelude: first score window needs k m-tile 0 (chunk 0) + q chunk 0
        qk_chain(wk_sb, bk_sb, k_sb, 0, 0)
        done.add(("qk", "k", 0, 0))
        qk_chain(wq_sb, bq_sb, q_sb, 0, 0)
        done.add(("qk", "q", 0, 0))
        for n in range(1, NC2):
            queue_qk("k", 0, n)

        # Anti-diagonal iteration order: row 0's V-projection overload and
        # row 3's output-projection load spread over interleaved iterations
        # of the other rows instead of saturating one row while ScalarE
        # starves.
        ORDER = [
            (p, s - p)
            for s in range(MQ + NC2 - 1)
            for p in range(max(0, s - NC2 + 1), min(MQ - 1, s) + 1)
        ]
        hist = []
        oproj_queued = set()
        for p, c in ORDER:
            # Queue this iteration's filler.  PV chains for the previous
            # pair are sandwiched between qkv-bank chains so the
            # single-buffered PSUM slots always have covering PE work
            # between their DVE drain and reuse.
            qkv_new = []
            if p == 0 and c < NC2 - 1:
                qkv_new.append(("q", 0, c + 1))
            if p < MQ - 1:
                qkv_new.append(("q", p + 1, c))
            # k for row s+1 (first used on the next diagonal) spread across
            # this diagonal's iterations; scores need a pair's k over ALL
            # key chunks at its first iteration
            s = p + c
            if s + 1 < MQ:
                d_lo = max(0, s - NC2 + 1)
                n_iters = min(MQ - 1, s) - d_lo + 1
                for n in range(NC2):
                    if n % n_iters == p - d_lo:
                        qkv_new.append(("k", s + 1, n))
            if hist:
                queue_pv_head(*hist[-1][:2], 0, hist[-1][2])
            if qkv_new:
                queue_qk(*qkv_new.pop(0))
            if hist:
                queue_pv_head(*hist[-1][:2], 1, hist[-1][2])
            for spec in qkv_new:
                queue_qk(*spec)
            if p == 0 and c == 0:
                queue_v()
            # output projection for any chunk whose last pair's PV (and thus
            # its o_sb transposes) is queued above or in an earlier iteration
            for cc in range(NC2):
                if cc not in oproj_queued and (MQ - 1, cc) in [h[:2] for h in hist]:
                    oproj_queued.add(cc)
                    queue_oproj(cc)
            pts = [
                ptp.tile([P, TK, TQC], bf16, tag="pt", name="pt0"),
                ptp.tile([P, TK, TQC], bf16, tag="pt", name="pt1"),
            ]
            g0 = pv_labels(*hist[-2][:2], 0) if len(hist) >= 2 else ()
            g1 = pv_labels(*hist[-2][:2], 1) if len(hist) >= 2 else ()
            # early iterations run ACT behind the input DMAs anyway, so the
            # windows can afford extra filler to pre-drain the V backlog
            boost = 1.4 if len(hist) < 6 else 1.0
            scores_exp_head(p, c, 0, pts[0], pt_guard=g0, boost=boost)
            if (p, c) == ORDER[-1]:
                # last iteration: its own head-0 PV can pump under the
                # head-1 score windows
                queue_pv_head(p, c, 0, pts)
            scores_exp_head(p, c, 1, pts[1], pt_guard=g1, boost=boost)
            hist.append((p, c, pts))

        # tail: last pair's PV, then the final output-projection chunk.
        # The first four chains park their ki 0-2 partial sums in the four
        # psum pools (score slots are free after the last exp) so they
        # execute under the final PV's normalize/transpose flight; only the
        # ki=3 matmuls wait for the last pair's o_sb transposes.
        p, c, pts = hist[-1]
        while fifo:
            pop_one()
        TAILP = ((qkvps, "qkv"), (scpsA, "scA"), (pvps, "pv"), (scpsB, "scB"))
        tq0 = (NC2 - 1) * TQC
        parked = [None] * 4

        def park(j):
            pool, tag = TAILP[j]
            ys = pool.tile([P, TQC], f32, tag=tag, name="ys")
            for ki in range(MQ - 1):
                nc.tensor.matmul(
                    ys,
                    wo_sb[:, ki, j * P : (j + 1) * P],
                    o_sb[:, ki, tq0 : tq0 + TQC],
                    start=(ki == 0),
                    stop=False,
                )
            parked[j] = ys

        # the qkvps-slot partial depends only on long-finished state: run it
        # under the last head's exp drain before the final PV chains
        park(0)
        for qs in range(MQ):
            pv_chain(p, c, 1, qs, pts[1])
        for j in (1, 2, 3):
            park(j)
        for j in range(4):
            ys = parked[j]
            nc.tensor.matmul(
                ys,
                wo_sb[:, MQ - 1, j * P : (j + 1) * P],
                o_sb[:, MQ - 1, tq0 : tq0 + TQC],
                start=False,
                stop=True,
            )
            yo = yop.tile([P, TQC], bf16, name="yo")
            if j % 2:
                nc.scalar.copy(yo, ys)
            else:
                nc.vector.tensor_copy(yo, ys)
            nc.sync.dma_start(out=yT[:, j, tq0 : tq0 + TQC], in_=yo)
        for j in range(4, D // P):
            pool, tag = TAILP[j % 4]
            o_chain(NC2 - 1, j, pool=pool, tag=tag, drain=("act" if j % 2 else None))

        if reps > 1:
            loop_cm.__exit__(None, None, None)

        if dbg is not None:
            nc.sync.dma_start(out=dbg["q"], in_=q_sb)
            nc.sync.dma_start(out=dbg["k"], in_=k_sb)
            nc.sync.dma_start(out=dbg["v"], in_=v_sb)
            nc.sync.dma_start(out=dbg["o"], in_=o_sb)


def _build(debug=False, reps=1):
    import concourse.tile as tile
    from concourse import bacc, mybir

    dt = mybir.dt
    f32, bf16 = dt.float32, dt.bfloat16

    f8 = dt.float8e4
    nc = bacc.Bacc("TRN2", target_bir_lowering=False, debug=False)
    # inputs are host-preswizzled into partition-major layouts so every DMA
    # descriptor is a fat contiguous run; x/wq/wk/wv ship as fp8 (hi, lo)
    # residual pairs (same byte volume as bf16)
    aps = {
        "xT": nc.dram_tensor("xT", [P, NC2, KD, 2, TQC], f8, kind="ExternalInput").ap(),
        "wq": nc.dram_tensor("wq", [P, MQ, KD, 2, P], f8, kind="ExternalInput").ap(),
        "wk": nc.dram_tensor("wk", [P, MQ, KD, 2, P], f8, kind="ExternalInput").ap(),
        "wv": nc.dram_tensor("wv", [P, KD, 2, DG], f8, kind="ExternalInput").ap(),
        "wo": nc.dram_tensor("wo", [P, MQ, D], bf16, kind="ExternalInput").ap(),
        "bq": nc.dram_tensor("bq", [P, MQ], f32, kind="ExternalInput").ap(),
        "bk": nc.dram_tensor("bk", [P, MQ], f32, kind="ExternalInput").ap(),
        "bv": nc.dram_tensor("bv", [1, DG], bf16, kind="ExternalInput").ap(),
        "yT": nc.dram_tensor("yT", [P, D // P, T], bf16, kind="ExternalOutput").ap(),
    }

    dbg = None
    if debug:
        dbg = {
            "q": nc.dram_tensor("dbg_q", [P, MQ, T], bf16, kind="ExternalOutput").ap(),
            "k": nc.dram_tensor("dbg_k", [P, MQ, T], bf16, kind="ExternalOutput").ap(),
            "v": nc.dram_tensor(
                "dbg_v", [P, TK, HG * VW], bf16, kind="ExternalOutput"
            ).ap(),
            "o": nc.dram_tensor("dbg_o", [P, MQ, T], bf16, kind="ExternalOutput").ap(),
            "pt": nc.dram_tensor(
                "dbg_pt", [P, TK, TQC], bf16, kind="ExternalOutput"
            ).ap(),
        }

    with tile.TileContext(nc) as tc:
        _emit(tc, aps, dbg, reps=reps)
    nc.compile()
    return nc


def _get_nc():
    if "nc" not in _CACHE:
        _CACHE["nc"] = _build()
    return _CACHE["nc"]


def _shard_inputs(x, Wq, bq, Wk, bk, Wv, bv, Wo, bo):
    import ml_dtypes

    bf16 = ml_dtypes.bfloat16
    f8 = ml_dtypes.float8_e4m3
    f32 = np.float32

    def c(a, dtype):
        return np.ascontiguousarray(a).astype(dtype)

    def kp(a, kt):  # [kt*P, F] -> [P, kt, F] partition-major swizzle
        return a.reshape(kt, P, a.shape[-1]).transpose(1, 0, 2)

    def hilo(a, order):  # [P, kt, F] f32 -> [P, kt, 2, F] fp8 residual pair
        hi = a.astype(f8)
        lo = (a - hi.astype(f32)).astype(f8)
        pair = {"hilo": (hi, lo), "lohi": (lo, hi)}[order]
        return np.ascontiguousarray(np.stack(pair, axis=2))

    def chunk_major(a8):  # [P, KD, 2, T] -> [P, NC2, KD, 2, TQC]
        return np.ascontiguousarray(
            a8.reshape(P, KD, 2, NC2, TQC).transpose(0, 3, 1, 2, 4)
        )

    def mtile_major(a8):  # [P, KD, 2, DG] -> [P, MQ, KD, 2, P]
        return np.ascontiguousarray(
            a8.reshape(P, KD, 2, MQ, P).transpose(0, 3, 1, 2, 4)
        )

    x8 = {}  # per-batch, shared by the two head-group cores
    in_maps = []
    for core in range(NCORES):
        b, g = core // 2, core % 2
        hs = g * DG
        if b not in x8:
            x8[b] = chunk_major(hilo(kp(np.asarray(x[b], dtype=f32).T, KD), "lohi"))
        in_maps.append(
            {
                "xT": x8[b],
                "wq": mtile_major(hilo(kp(Wq[hs : hs + DG, :].T * WS, KD), "hilo")),
                "wk": mtile_major(hilo(kp(Wk[hs : hs + DG, :].T * WS, KD), "hilo")),
                "wv": hilo(kp(Wv[hs : hs + DG, :].T * WS, KD), "hilo"),
                "wo": c(kp(Wo[:, hs : hs + DG].T, MQ), bf16),
                "bq": c(bq[hs : hs + DG].reshape(MQ, P).T * WS, f32),
                "bk": c(bk[hs : hs + DG].reshape(MQ, P).T * WS, f32),
                "bv": c(bv[hs : hs + DG].reshape(1, DG) * WS, bf16),
            }
        )
    return in_maps


def _run(inputs, trace=False):
    from concourse import bass_utils

    nc = _get_nc()
    np_in = {k: np.asarray(v) for k, v in inputs.items()}
    in_maps = _shard_inputs(**np_in)
    res = bass_utils.run_bass_kernel_spmd(
        nc, in_maps, core_ids=list(range(NCORES)), trace=trace
    )
    bo = np_in["bo"].astype(np.float32)
    y = np.empty((B, T, D), dtype=np.float32)
    for b in range(B):
        acc = res.results[2 * b]["yT"].astype(np.float32) + res.results[
            2 * b + 1
        ]["yT"].astype(np.float32)  # [P, D/P, T]
        y[b] = acc.transpose(1, 0, 2).reshape(D, T).T + bo
    return y, res


def kernel(**inputs):
    y, _ = _run(inputs)
    return y



# revision 37
# speedup vs baseline: 1.0309x; 1.0131x over previous
"""Multi-head attention (B=4, T=2048, D=1024, H=16) on 8 Trainium2 NeuronCores.

Sharding: core = (batch, head-group): b = core // 2, g = core % 2.
Each core computes heads [g*8, g*8+8) of batch b:
  - Q/K/V projections as fp8e4m3 DoubleRow residual matmuls: x and W ship as
    host-quantized (hi, lo) pairs; hi@hi runs over k-tile pairs and one
    cross-term instruction per k-tile adds w_hi@x_lo + w_lo@x_hi (only lo@lo
    is dropped), at 0.75x the bf16 PE cost and ~bf16 accuracy.  W is
    prescaled by 64 so its lo parts clear e4m3's subnormal floor; the scale
    is undone in the exp (q,k) and the V psum drain (v).
  - scores transposed: S.T tile = K_h @ Q_h.T (bf16, K=64) into alternating
    3-bank A/B PSUM slots; exp on ScalarE per [128, 3, 512] window
    (plus one 1-kt window), scale = 1/(sqrt(64)*64^2), no max subtraction
  - PV flipped: lhsT = P^T subtile (stationary), rhs = [V_h | 1] (moving,
    N=65) -> O natural [q, hd] per 128-query subtile with row-sums in col
    64; normalize on DVE (per-partition scalar), XBAR DMA transpose back to
    o_sb's [hd, q] layout (no PE broadcast/transpose rows)
  - partial output projection yT_g = Wo[:, g].T-contraction, bf16 out
Host: y[b] = (yT_part[2b] + yT_part[2b+1]).T + bo.

Scheduling: ScalarE's exp stream is the near-bottleneck, so score windows
are emitted at ACT's drain rate and every other PE chain (projections, PV,
output projection) is queued as a small piece popped between windows under
a credit budget.  Iterations walk (pair, chunk) anti-diagonals so the V
projection (early) and the per-chunk output projections (late) spread over
interleaved iterations; single-buffered PSUM slots alternate users so
chain drains overlap; the final output chunk parks its ki 0-2 partials in
the freed score slots while the last PV normalizes.

Self-contained: hardcodes all shapes; requires only concourse (bass) + numpy.
"""

import numpy as np

B, T, D = 4, 2048, 1024
H, HD = 16, 64
HG, DG = 8, 512          # heads / feature columns per core
NCORES = 8
P = 128
KD = D // P              # 8  k-tiles over model dim
MQ = DG // P             # 4  partition tiles of qT/kT/oT (one per head pair)
TK = T // P              # 16 key tiles
TQC = 512                # query-chunk (= one fp32 PSUM bank)
NC2 = T // TQC           # 4  query chunks
VW = HD + 1              # V columns per head incl. ones column
SCALE = 0.125            # 1/sqrt(HD)
WS = 64.0                # host-side weight prescale: keeps the fp8 residual
                         # (lo) parts of W ~ N(0, 1/1024) above e4m3's
                         # subnormal floor; undone via the exp scale (q,k)
                         # and the V psum->sbuf copy
SCALE_E = SCALE / (WS * WS)

_CACHE: dict = {}


def _emit(tc, aps, dbg=None, reps=1):
    import concourse.bass as bass  # noqa: F401
    from concourse import mybir

    nc = tc.nc
    dt = mybir.dt
    f32, bf16 = dt.float32, dt.bfloat16
    AF = mybir.ActivationFunctionType
    xT, wq, wk, wv, wo, bq, bk, bvr, ident, yT = (
        aps["xT"], aps["wq"], aps["wk"], aps["wv"], aps["wo"],
        aps["bq"], aps["bk"], aps["bvr"], aps["ident"], aps["yT"],
    )

    from collections import deque
    from contextlib import ExitStack

    with ExitStack() as ctx:
        const = ctx.enter_context(tc.tile_pool(name="const", bufs=1))
        persist = ctx.enter_context(tc.tile_pool(name="persist", bufs=1))
        xw = ctx.enter_context(tc.tile_pool(name="xw", bufs=1))
        ptp = ctx.enter_context(tc.tile_pool(name="ptp", bufs=5))
        opp = ctx.enter_context(tc.tile_pool(name="opp", bufs=2))
        yop = ctx.enter_context(tc.tile_pool(name="yop", bufs=3))
        nrm = ctx.enter_context(tc.tile_pool(name="nrm", bufs=4))
        # PSUM: 3(A) + 3(B) + 1(proj) + 1(pv) = 8 banks exactly
        scpsA = ctx.enter_context(tc.tile_pool(name="scpsA", bufs=1, space="PSUM"))
        scpsB = ctx.enter_context(tc.tile_pool(name="scpsB", bufs=1, space="PSUM"))
        qkvps = ctx.enter_context(tc.tile_pool(name="qkvps", bufs=1, space="PSUM"))
        pvps = ctx.enter_context(tc.tile_pool(name="pvps", bufs=1, space="PSUM"))

        # ---- persistent SBUF ----
        # q is double-buffered over chunks (a pair's q(c) is only read during
        # iteration (p, c), and chains are queued at most one iteration
        # ahead); the freed 8KB/partition funds the 5th pt slot.
        q_sb = persist.tile([P, MQ, 2, TQC], bf16)
        k_sb = persist.tile([P, MQ, T], bf16)
        v_sb = persist.tile([P, TK, HG * VW], bf16)
        o_sb = persist.tile([P, MQ, T], bf16)
        v4d = v_sb.rearrange("p t (h c) -> p t h c", h=HG)
        nc.vector.memset(v4d[:, :, :, HD : HD + 1], 1.0)

        # ---- input DMAs ----------------------------------------------------
        # x and the q/k/v weights arrive as fp8 (hi, lo) residual pairs for
        # DoubleRow matmuls.  x dim2 order is (lo, hi); w dim2 is (hi, lo):
        # the cross-term DoubleRow instruction then contracts
        # w_hi.T@x_lo + w_lo.T@x_hi with natural slices.
        f8 = dt.float8e4
        x_sb = xw.tile([P, NC2, KD, 2, TQC], f8)
        wq_sb = xw.tile([P, MQ, KD, 2, P], f8)
        wk_sb = xw.tile([P, MQ, KD, 2, P], f8)
        wv_sb = xw.tile([P, KD, 2, DG], f8)
        bvr_sb = xw.tile([P, DG], bf16)
        bvr4d = bvr_sb.rearrange("p (h c) -> p h c", h=HG)
        wo_sb = const.tile([P, MQ, D], bf16)
        bq_sb = const.tile([P, MQ], f32)
        bk_sb = const.tile([P, MQ], f32)
        ident_sb = const.tile([P, P], bf16)
        # Single queue, critical-path order (transfers serialize at aggregate
        # HBM bandwidth even across the two HWDGE queues, so order is
        # everything): wk/wq m-tile 0 + biases, then x chunk 0 in 4 kt-pair
        # pieces so the prelude k/q chains STREAM against the DMA (each
        # piece unlocks 6 matmuls), then the remaining x slabs one window
        # group ahead of the k chains that need them.
        nc.sync.dma_start(out=wk_sb[:, 0], in_=wk[0])
        nc.sync.dma_start(out=wq_sb[:, 0], in_=wq[0])
        for kp in range(KD // 2):
            nc.sync.dma_start(
                out=x_sb[:, 0, 2 * kp : 2 * kp + 2], in_=xT[:, 0, 2 * kp : 2 * kp + 2]
            )
        nc.sync.dma_start(out=bk_sb, in_=bk)
        nc.sync.dma_start(out=bq_sb, in_=bq)
        nc.sync.dma_start(out=x_sb[:, 1], in_=xT[:, 1])
        nc.sync.dma_start(out=wv_sb, in_=wv)
        nc.sync.dma_start(out=bvr_sb, in_=bvr)
        nc.sync.dma_start(out=x_sb[:, 2], in_=xT[:, 2])
        nc.sync.dma_start(out=x_sb[:, 3], in_=xT[:, 3])
        for mt in range(1, MQ):
            nc.sync.dma_start(out=wk_sb[:, mt], in_=wk[mt])
        for mt in range(1, MQ):
            nc.sync.dma_start(out=wq_sb[:, mt], in_=wq[mt])
        nc.sync.dma_start(out=wo_sb, in_=wo)
        nc.sync.dma_start(out=ident_sb, in_=ident)
        ones_sb = xw.tile([1, P], bf16)
        nc.vector.memset(ones_sb, 1.0)
        # dummy exp: binds the one-time ACT function-table load (1.3us) to an
        # instruction with no data deps so it runs at t~0 instead of delaying
        # the prelude's k-bias on the first-exp critical path
        dummy_sb = xw.tile([1, 1], f32)
        nc.scalar.activation(dummy_sb, ones_sb[0:1, 0:1], AF.Exp, scale=1.0)
        DR = mybir.MatmulPerfMode.DoubleRow

        # p-state warmup: keep the PE continuously busy with junk matmuls
        # until the first projection chain's inputs land, so real work runs
        # at full clock instead of through the p-state ramp.
        warm_ps = qkvps.tile([P, HD], f32, tag="qkv", name="warm")
        for _ in range(45):
            nc.tensor.matmul(warm_ps, ones_sb, ones_sb[:, 0:HD], start=True, stop=True)

        # ---- filler FIFO + credit pump -------------------------------------
        # The exp() stream on ScalarE is the near-bottleneck; score windows
        # are emitted at ACT's drain rate and all other PE work (projection
        # chains, PV chains, output-projection chains) is queued as small
        # "filler" pieces popped between windows so the PE never waits on a
        # PSUM slot while ACT catches up.
        fifo = deque()  # (label, est_pe_ns, emit_fn)
        done = set()
        state = {"credit": 0.0}

        def piece(label, est, fn):
            fifo.append((label, est, fn))

        def pop_one():
            label, est, fn = fifo.popleft()
            fn()
            done.add(label)
            state["credit"] -= est

        def pump(add):
            state["credit"] = min(state["credit"] + add, 2600.0)
            while fifo and state["credit"] > 0:
                pop_one()

        def need(*labels):
            """Emit required pieces.  qk chains depend on nothing queued
            before them (DMA inputs, disjoint outputs), so they may jump the
            queue instead of dragging the whole FIFO prefix into a score
            window; anything else drains in order."""
            want = set(labels) - done
            for lbl in [w for w in want if w[0] == "qk"]:
                for idx, (l2, est, fn) in enumerate(fifo):
                    if l2 == lbl:
                        del fifo[idx]
                        fn()
                        done.add(lbl)
                        state["credit"] -= est
                        break
            want -= done
            while want:
                assert fifo, f"missing pieces: {want}"
                pop_one()
                want -= done

        # ---- work pieces ---------------------------------------------------
        def qk_chain(which, mt, n):
            """fp8 DoubleRow residual projection: hi@hi over k-tile pairs,
            then per-k-tile cross terms (w_hi@x_lo + w_lo@x_hi); only the
            lo@lo term is dropped (~1e-3 relative)."""
            w_sb, b_col, dst = {
                "k": (wk_sb, bk_sb, k_sb[:, mt, n * TQC : (n + 1) * TQC]),
                "q": (wq_sb, bq_sb, q_sb[:, mt, n % 2]),
            }[which]
            ps = qkvps.tile([P, TQC], f32, tag="qkv", name="ps_qkv")
            for kp in range(KD // 2):
                nc.tensor.matmul(
                    ps,
                    w_sb[:, mt, 2 * kp : 2 * kp + 2, 0, :],
                    x_sb[:, n, 2 * kp : 2 * kp + 2, 1, :],
                    start=(kp == 0),
                    stop=False,
                    perf_mode=DR,
                )
            for ki in range(KD):
                nc.tensor.matmul(
                    ps,
                    w_sb[:, mt, ki, :, :],
                    x_sb[:, n, ki, :, :],
                    start=False,
                    stop=(ki == KD - 1),
                    perf_mode=DR,
                )
            nc.vector.tensor_scalar_add(dst, ps, b_col[:, mt : mt + 1])

        def queue_qk(which, mt, n):
            piece(
                ("qk", which, mt, n),
                1280,
                lambda which=which, mt=mt, n=n: qk_chain(which, mt, n),
            )

        def v_chain(t):
            """DoubleRow residual scheme with x stationary; the psum->sbuf
            copy divides out the host-side weight prescale and adds the bias
            (host-replicated across partitions) in the same DVE op.  Chains
            alternate between the two 1-bank psum pools so consecutive
            chains overlap each other's DVE drain."""
            pool, tg = ((qkvps, "qkv"), (pvps, "pv"))[t % 2]
            ps = pool.tile([P, DG], f32, tag=tg, name="ps_v")
            n, ts = t // (TQC // P), (t % (TQC // P)) * P
            for kp in range(KD // 2):
                nc.tensor.matmul(
                    ps,
                    x_sb[:, n, 2 * kp : 2 * kp + 2, 1, ts : ts + P],
                    wv_sb[:, 2 * kp : 2 * kp + 2, 0, :],
                    start=(kp == 0),
                    stop=False,
                    perf_mode=DR,
                )
            for ki in range(KD):
                nc.tensor.matmul(
                    ps,
                    x_sb[:, n, ki, :, ts : ts + P],
                    wv_sb[:, ki, :, :],
                    start=False,
                    stop=(ki == KD - 1),
                    perf_mode=DR,
                )
            nc.vector.scalar_tensor_tensor(
                v4d[:, t, :, 0:HD],
                ps.rearrange("p (h c) -> p h c", h=HG),
                1.0 / WS,
                bvr4d,
                mybir.AluOpType.mult,
                mybir.AluOpType.add,
            )

        def queue_v():
            for t in range(TK):
                piece(("v", t), 1290, lambda t=t: v_chain(t))

        # Flipped PV for head h = 2p + i: lhsT = P^T subtile (stationary),
        # rhs = [V_h | 1] (moving, N = 65) so each 128-query subtile
        # accumulates O natural [q, hd] plus its softmax row-sums in column
        # 64.  Normalize on DVE (row-sums live on the free dim); after both
        # heads, XBAR DMA transposes put O back into o_sb's [hd, q] layout
        # without touching the PE.
        pvstate = {}

        def pv_chain(p, c, i, qs, pt):
            h = 2 * p + i
            if (i, qs) == (0, 0):
                pvstate["opr"] = opp.tile([P, MQ, 2, HD], bf16, name="opair")
            if qs == 0:
                pvstate["pv"] = pvps.tile([P, MQ, VW], f32, tag="pv", name="pv")
            pv = pvstate["pv"]
            for tk in range(TK):
                nc.tensor.matmul(
                    pv[:, qs, :],
                    pt[:, tk, qs * P : (qs + 1) * P],
                    v_sb[:, tk, h * VW : (h + 1) * VW],
                    start=(tk == 0),
                    stop=(tk == TK - 1),
                )
            opr = pvstate["opr"]
            rc = nrm.tile([P, 1], f32, name="rc")
            nc.vector.reciprocal(rc, pv[:, qs, HD : HD + 1])
            nc.vector.tensor_scalar_mul(opr[:, qs, i, :], pv[:, qs, 0:HD], rc)
            if i == 1:
                # this query-subtile now has both heads normalized: transpose
                # it back immediately rather than after the whole head drains
                tq0 = c * TQC
                nc.sync.dma_start_transpose(
                    out=o_sb[:, p, tq0 + qs * P : tq0 + (qs + 1) * P],
                    in_=opr[:, qs, :, :],
                )

        def pv_labels(p, c, i):
            return [("pv", p, c, i, qs) for qs in range(MQ)]

        def queue_pv_head(p, c, i, pts):
            for qs in range(MQ):
                piece(
                    ("pv", p, c, i, qs),
                    433,
                    lambda p=p, c=c, i=i, qs=qs, pt=pts[i]: pv_chain(p, c, i, qs, pt),
                )

        def o_chain(c, j, pool=None, tag=None, drain=None):
            tq0 = c * TQC
            if pool is None:
                pool, tag = ((qkvps, "qkv"), (pvps, "pv"))[j % 2]
            ys = pool.tile([P, TQC], f32, tag=tag, name="ys")
            for ki in range(MQ):
                nc.tensor.matmul(
                    ys,
                    wo_sb[:, ki, j * P : (j + 1) * P],
                    o_sb[:, ki, tq0 : tq0 + TQC],
                    start=(ki == 0),
                    stop=(ki == MQ - 1),
                )
            yo = yop.tile([P, TQC], bf16, name="yo")
            if drain == "act":
                nc.scalar.copy(yo, ys)
            else:
                nc.vector.tensor_copy(yo, ys)
            nc.sync.dma_start(out=yT[:, j, tq0 : tq0 + TQC], in_=yo)

        def queue_oproj(c):
            for j in range(D // P):
                piece(("oproj", c, j), 853, lambda c=c, j=j: o_chain(c, j))

        # ---- score windows -------------------------------------------------
        # Two 3-bank PSUM slots, windows [3,3,3,3,2,2] strictly alternating
        # A/B: larger exp() instructions amortize ScalarE's fixed
        # per-instruction cost and no window ever reuses the slot of the
        # immediately preceding one.  The two trailing 2-tile windows keep
        # every window's ACT cover >= ~1040ns: a pool slot's next refill
        # needs sem + fill + sem (~890ns) after its previous exp ends, so a
        # trailing 1-tile window (612ns cover) would stall ACT ~380ns at
        # every head boundary.
        WINDOWS = (
            (scpsA, "scA", 3, 825.0),
            (scpsB, "scB", 3, 825.0),
            (scpsA, "scA", 3, 825.0),
            (scpsB, "scB", 3, 825.0),
            (scpsA, "scA", 2, 612.0),
            (scpsB, "scB", 2, 612.0),
        )

        def scores_exp_head(p, c, i, pt, pt_guard=(), boost=1.0):
            hb = i * HD
            kt0 = 0
            for pool, tg, wn, credit in WINDOWS:
                n_lo = (kt0 * P) // TQC
                n_hi = ((kt0 + wn) * P - 1) // TQC
                need(
                    *[("qk", "k", p, n2) for n2 in range(n_lo, n_hi + 1)],
                    ("qk", "q", p, c),
                )
                scs = pool.tile([P, wn, TQC], f32, tag=tg, name=tg)
                for u in range(wn):
                    tk = kt0 + u
                    nc.tensor.matmul(
                        scs[:, u, :],
                        k_sb[hb : hb + HD, p, tk * P : (tk + 1) * P],
                        q_sb[hb : hb + HD, p, c % 2],
                        start=True,
                        stop=True,
                    )
                if kt0 == 0 and pt_guard:
                    # the exp below reuses the pt slot read by a PV two
                    # iterations back: force that PV out after this window's
                    # matmuls (which don't touch pt) rather than before
                    need(*pt_guard)
                nc.scalar.activation(pt[:, kt0 : kt0 + wn, :], scs, AF.Exp, scale=SCALE_E)
                kt0 += wn
                pump(credit * boost)

        # ---- schedule: pair-outer, chunk-inner -----------------------------
        if reps > 1:
            loop_cm = tc.For_i(0, reps, 1)
            loop_cm.__enter__()

        fifo.clear()
        done.clear()
        state["credit"] = 0.0

        # prelude: first score window needs k m-tile 0 (chunk 0) + q chunk 0.
        # The k and q chains interleave per x-chunk-0 kt-pair piece so each
        # piece's 6 matmuls run while the next piece's DMA is in flight; the
        # k bias drains on ACT (idle pre-first-exp) and q's on DVE so the two
        # psum->sbuf copies overlap.
        ps_k = qkvps.tile([P, TQC], f32, tag="qkv", name="ps_qkv")
        ps_q = pvps.tile([P, TQC], f32, tag="pv", name="ps_v")
        for kp in range(KD // 2):
            for w_sb, ps in ((wk_sb, ps_k), (wq_sb, ps_q)):
                nc.tensor.matmul(
                    ps,
                    w_sb[:, 0, 2 * kp : 2 * kp + 2, 0, :],
                    x_sb[:, 0, 2 * kp : 2 * kp + 2, 1, :],
                    start=(kp == 0),
                    stop=False,
                    perf_mode=DR,
                )
                for ki in (2 * kp, 2 * kp + 1):
                    nc.tensor.matmul(
                        ps,
                        w_sb[:, 0, ki, :, :],
                        x_sb[:, 0, ki, :, :],
                        start=False,
                        stop=(ki == KD - 1),
                        perf_mode=DR,
                    )
        nc.scalar.activation(
            k_sb[:, 0, 0:TQC], ps_k, AF.Identity, bias=bk_sb[:, 0:1], scale=1.0
        )
        nc.vector.tensor_scalar_add(q_sb[:, 0, 0], ps_q, bq_sb[:, 0:1])
        done.add(("qk", "k", 0, 0))
        done.add(("qk", "q", 0, 0))
        for n in range(1, NC2):
            queue_qk("k", 0, n)

        # Anti-diagonal iteration order: row 0's V-projection overload and
        # row 3's output-projection load spread over interleaved iterations
        # of the other rows instead of saturating one row while ScalarE
        # starves.
        # Within late diagonals (s >= 4), visit pairs in DESCENDING p order so
        # each (3, c) iteration lands one diagonal earlier: its PV chains and
        # oproj(c) then pump under the next diagonal's windows instead of
        # piling into the tail.  Early diagonals stay ascending (pair 0's
        # k/q/V availability paces the start).
        ORDER = []
        for s in range(MQ + NC2 - 1):
            diag = [
                (p, s - p)
                for p in range(max(0, s - NC2 + 1), min(MQ - 1, s) + 1)
            ]
            ORDER.extend(diag if s < 4 else diag[::-1])
        hist = []
        pt_state = {"idx": 0}
        oproj_queued = set()
        for p, c in ORDER:
            # Queue this iteration's filler.  PV chains for the previous
            # pair are sandwiched between qkv-bank chains so the
            # single-buffered PSUM slots always have covering PE work
            # between their DVE drain and reuse.
            qkv_new = []
            if p == 0 and c < NC2 - 1:
                qkv_new.append(("q", 0, c + 1))
            if p < MQ - 1:
                qkv_new.append(("q", p + 1, c))
            # k for row s+1 (first used on the next diagonal) spread across
            # this diagonal's iterations; scores need a pair's k over ALL
            # key chunks at its first iteration
            s = p + c
            if s + 1 < MQ:
                d_lo = max(0, s - NC2 + 1)
                n_iters = min(MQ - 1, s) - d_lo + 1
                for n in range(NC2):
                    if n % n_iters == p - d_lo:
                        qkv_new.append(("k", s + 1, n))
            if hist:
                queue_pv_head(*hist[-1][:2], 0, hist[-1][2])
            if qkv_new:
                queue_qk(*qkv_new.pop(0))
            if hist:
                queue_pv_head(*hist[-1][:2], 1, hist[-1][2])
            for spec in qkv_new:
                queue_qk(*spec)
            if p == 0 and c == 0:
                queue_v()
            # output projection for any chunk whose last pair's PV (and thus
            # its o_sb transposes) is queued above or in an earlier iteration
            for cc in range(NC2):
                if cc not in oproj_queued and (MQ - 1, cc) in [h[:2] for h in hist]:
                    oproj_queued.add(cc)
                    queue_oproj(cc)
            pts = [
                ptp.tile([P, TK, TQC], bf16, tag="pt", name="pt0"),
                ptp.tile([P, TK, TQC], bf16, tag="pt", name="pt1"),
            ]
            # pt slots rotate round-robin over the pool's 5 bufs (2 allocs
            # per iteration); an exp may only overwrite a slot whose previous
            # owner's PV chains are already emitted, so guard each head's
            # first window with the evicted owner's PV labels.
            guards = []
            for i in (0, 1):
                slot = pt_state["idx"] % 5
                guards.append(
                    pv_labels(*pt_state[slot]) if slot in pt_state else ()
                )
                pt_state[slot] = (p, c, i)
                pt_state["idx"] += 1
            g0, g1 = guards
            # early iterations run ACT behind the input DMAs anyway, so the
            # windows can afford extra filler to pre-drain the V backlog
            boost = 1.4 if len(hist) < 6 else 1.0
            scores_exp_head(p, c, 0, pts[0], pt_guard=g0, boost=boost)
            if (p, c) == ORDER[-1]:
                # last iteration: its own head-0 PV can pump under the
                # head-1 score windows
                queue_pv_head(p, c, 0, pts)
            scores_exp_head(p, c, 1, pts[1], pt_guard=g1, boost=boost)
            hist.append((p, c, pts))

        # tail: last pair's head-1 PV, then the final output-projection
        # chunk.  ALL EIGHT j-columns park their ki 0-2 partial sums across
        # the freed psum banks (3 in each score slot + the two 1-bank pools)
        # so they execute under the final exp drain / PV normalize flight;
        # only the 8 ki=3 matmuls wait for the last pair's o_sb transposes.
        # The PV chains alternate between the two 1-bank pools so a chain
        # never waits for the previous chain's normalize to release its bank,
        # and their transposes split across both HWDGE queues (ACT's queue is
        # idle here) so the 4 dispatches pipeline two-wide.
        p, c, pts = hist[-1]
        while fifo:
            pop_one()
        tq0 = (NC2 - 1) * TQC
        parkA = scpsA.tile([P, 3, TQC], f32, tag="scA", name="parkA")
        parkB = scpsB.tile([P, 3, TQC], f32, tag="scB", name="parkB")
        slots = [parkA[:, 0], parkA[:, 1], parkA[:, 2],
                 parkB[:, 0], parkB[:, 1], parkB[:, 2], None, None]

        def park(j, ki_hi=MQ - 1):
            for ki in range(ki_hi):
                nc.tensor.matmul(
                    slots[j],
                    wo_sb[:, ki, j * P : (j + 1) * P],
                    o_sb[:, ki, tq0 : tq0 + TQC],
                    start=(ki == 0),
                    stop=False,
                )

        # parkA depends on the second-to-last exp window's slot, parkB on the
        # last one; both run under the tail exp/PV flight.
        for j in range(6):
            park(j)
        # Tail PV: chains alternate the two 1-bank pools so a chain never
        # waits the previous chain's normalize, and the o_sb transposes go
        # through the PE (identity transpose into psum + DVE copy) instead
        # of XBAR DMA: on-chip sems are ~10x cheaper than the DMA path's
        # dispatch + 900ns completion sem.
        opr = pvstate["opr"]
        pvt = [None, None]
        for qs in range(MQ):
            pool, tg = ((pvps, "pv"), (qkvps, "qkv"))[qs % 2]
            if qs < 2:
                pvt[qs] = pool.tile([P, 2, VW], f32, tag=tg, name="pvt")
            pv = pvt[qs % 2]
            for tk in range(TK):
                nc.tensor.matmul(
                    pv[:, qs // 2, :],
                    pts[1][:, tk, qs * P : (qs + 1) * P],
                    v_sb[:, tk, (2 * p + 1) * VW : (2 * p + 2) * VW],
                    start=(tk == 0),
                    stop=(tk == TK - 1),
                )
            rc = nrm.tile([P, 1], f32, name="rc")
            nc.vector.reciprocal(rc, pv[:, qs // 2, HD : HD + 1])
            nc.vector.tensor_scalar_mul(opr[:, qs, 1, :], pv[:, qs // 2, 0:HD], rc)
        tp_ps = pvps.tile([P, MQ, P], bf16, tag="pv", name="tp_ps")
        for qs in range(MQ):
            nc.tensor.transpose(tp_ps[:, qs, :], opr[:, qs, :, :], ident_sb)
            nc.vector.tensor_copy(
                o_sb[:, p, tq0 + qs * P : tq0 + (qs + 1) * P], tp_ps[:, qs, :]
            )
        slots[6] = qkvps.tile([P, TQC], f32, tag="qkv", name="ys6")
        park(6)
        slots[7] = pvps.tile([P, TQC], f32, tag="pv", name="ys7")
        park(7)
        ytail = ptp.tile([P, D // P, TQC], bf16, tag="pt", name="ytail")
        for j in range(D // P):
            nc.tensor.matmul(
                slots[j],
                wo_sb[:, MQ - 1, j * P : (j + 1) * P],
                o_sb[:, MQ - 1, tq0 : tq0 + TQC],
                start=False,
                stop=True,
            )
        for j in range(D // P):
            if j % 2:
                nc.scalar.copy(ytail[:, j], slots[j])
            else:
                nc.vector.tensor_copy(ytail[:, j], slots[j])
            if j % 2:
                nc.sync.dma_start(
                    out=yT[:, j - 1 : j + 1, tq0 : tq0 + TQC],
                    in_=ytail[:, j - 1 : j + 1],
                )

        if reps > 1:
            loop_cm.__exit__(None, None, None)

        if dbg is not None:
            nc.sync.dma_start(out=dbg["q"], in_=q_sb)
            nc.sync.dma_start(out=dbg["k"], in_=k_sb)
            nc.sync.dma_start(out=dbg["v"], in_=v_sb)
            nc.sync.dma_start(out=dbg["o"], in_=o_sb)


def _build(debug=False, reps=1):
    import concourse.tile as tile
    from concourse import bacc, mybir

    dt = mybir.dt
    f32, bf16 = dt.float32, dt.bfloat16

    f8 = dt.float8e4
    nc = bacc.Bacc("TRN2", target_bir_lowering=False, debug=False)
    # inputs are host-preswizzled into partition-major layouts so every DMA
    # descriptor is a fat contiguous run; x/wq/wk/wv ship as fp8 (hi, lo)
    # residual pairs (same byte volume as bf16)
    aps = {
        "xT": nc.dram_tensor("xT", [P, NC2, KD, 2, TQC], f8, kind="ExternalInput").ap(),
        "wq": nc.dram_tensor("wq", [MQ, P, KD, 2, P], f8, kind="ExternalInput").ap(),
        "wk": nc.dram_tensor("wk", [MQ, P, KD, 2, P], f8, kind="ExternalInput").ap(),
        "wv": nc.dram_tensor("wv", [P, KD, 2, DG], f8, kind="ExternalInput").ap(),
        "wo": nc.dram_tensor("wo", [P, MQ, D], bf16, kind="ExternalInput").ap(),
        "bq": nc.dram_tensor("bq", [P, MQ], f32, kind="ExternalInput").ap(),
        "bk": nc.dram_tensor("bk", [P, MQ], f32, kind="ExternalInput").ap(),
        "bvr": nc.dram_tensor("bvr", [P, DG], bf16, kind="ExternalInput").ap(),
        "ident": nc.dram_tensor("ident", [P, P], bf16, kind="ExternalInput").ap(),
        "yT": nc.dram_tensor("yT", [P, D // P, T], bf16, kind="ExternalOutput").ap(),
    }

    dbg = None
    if debug:
        dbg = {
            "q": nc.dram_tensor(
                "dbg_q", [P, MQ, 2, TQC], bf16, kind="ExternalOutput"
            ).ap(),
            "k": nc.dram_tensor("dbg_k", [P, MQ, T], bf16, kind="ExternalOutput").ap(),
            "v": nc.dram_tensor(
                "dbg_v", [P, TK, HG * VW], bf16, kind="ExternalOutput"
            ).ap(),
            "o": nc.dram_tensor("dbg_o", [P, MQ, T], bf16, kind="ExternalOutput").ap(),
            "pt": nc.dram_tensor(
                "dbg_pt", [P, TK, TQC], bf16, kind="ExternalOutput"
            ).ap(),
        }

    with tile.TileContext(nc) as tc:
        _emit(tc, aps, dbg, reps=reps)
    nc.compile()
    return nc


def _get_nc():
    if "nc" not in _CACHE:
        _CACHE["nc"] = _build()
    return _CACHE["nc"]


def _shard_inputs(x, Wq, bq, Wk, bk, Wv, bv, Wo, bo):
    import ml_dtypes

    bf16 = ml_dtypes.bfloat16
    f8 = ml_dtypes.float8_e4m3
    f32 = np.float32

    def c(a, dtype):
        return np.ascontiguousarray(a).astype(dtype)

    def kp(a, kt):  # [kt*P, F] -> [P, kt, F] partition-major swizzle
        return a.reshape(kt, P, a.shape[-1]).transpose(1, 0, 2)

    def hilo(a, order):  # [P, kt, F] f32 -> [P, kt, 2, F] fp8 residual pair
        hi = a.astype(f8)
        lo = (a - hi.astype(f32)).astype(f8)
        pair = {"hilo": (hi, lo), "lohi": (lo, hi)}[order]
        return np.ascontiguousarray(np.stack(pair, axis=2))

    def chunk_major(a8):  # [P, KD, 2, T] -> [P, NC2, KD, 2, TQC]
        return np.ascontiguousarray(
            a8.reshape(P, KD, 2, NC2, TQC).transpose(0, 3, 1, 2, 4)
        )

    def mtile_major(a8):  # [P, KD, 2, DG] -> [MQ, P, KD, 2, P]
        return np.ascontiguousarray(
            a8.reshape(P, KD, 2, MQ, P).transpose(3, 0, 1, 2, 4)
        )

    x8 = {}  # per-batch, shared by the two head-group cores
    in_maps = []
    for core in range(NCORES):
        b, g = core // 2, core % 2
        hs = g * DG
        if b not in x8:
            x8[b] = chunk_major(hilo(kp(np.asarray(x[b], dtype=f32).T, KD), "lohi"))
        in_maps.append(
            {
                "xT": x8[b],
                "wq": mtile_major(hilo(kp(Wq[hs : hs + DG, :].T * WS, KD), "hilo")),
                "wk": mtile_major(hilo(kp(Wk[hs : hs + DG, :].T * WS, KD), "hilo")),
                "wv": hilo(kp(Wv[hs : hs + DG, :].T * WS, KD), "hilo"),
                "wo": c(kp(Wo[:, hs : hs + DG].T, MQ), bf16),
                "bq": c(bq[hs : hs + DG].reshape(MQ, P).T * WS, f32),
                "bk": c(bk[hs : hs + DG].reshape(MQ, P).T * WS, f32),
                "bvr": c(np.broadcast_to(bv[hs : hs + DG], (P, DG)), bf16),
                "ident": c(np.eye(P), bf16),
            }
        )
    return in_maps


def _run(inputs, trace=False):
    from concourse import bass_utils

    nc = _get_nc()
    np_in = {k: np.asarray(v) for k, v in inputs.items()}
    in_maps = _shard_inputs(**np_in)
    res = bass_utils.run_bass_kernel_spmd(
        nc, in_maps, core_ids=list(range(NCORES)), trace=trace
    )
    bo = np_in["bo"].astype(np.float32)
    y = np.empty((B, T, D), dtype=np.float32)
    for b in range(B):
        acc = res.results[2 * b]["yT"].astype(np.float32) + res.results[
            2 * b + 1
        ]["yT"].astype(np.float32)  # [P, D/P, T]
        y[b] = acc.transpose(1, 0, 2).reshape(D, T).T + bo
    return y, res


def kernel(**inputs):
    y, _ = _run(inputs)
    return y

